# revision 25
# baseline (speedup 1.0000x reference)
# BertSelfAttention Trainium2 Bass kernel.
#
# Problem: B=4, S=2048, HID=1024, NH=16, HD=64, fp32.
#   out = softmax((X Wq + bq)(X Wk + bk)^T / sqrt(HD) + mask) (X Wv + bv)
#
# Sharding (8 cores): data-parallel over B (4) x tensor-parallel over the 16
# heads (2 halves of 8 heads = 512 columns of Wq/Wk/Wv). core = b*2 + half.
# No cross-core communication; each core computes attention for its 8 heads
# and writes out[b, :, half*512:(half+1)*512].
#
# Per-core algorithm. All attention-stage operands (Q^T, K^T, exp(scores),
# V_aug) are stored fp16: the PE streams 16-bit moving operands at 1
# column/cycle vs 2 cycles/column for fp32/f32r, halving matmul time; PSUM
# accumulation stays fp32 so softmax denominators and ctx sums are exact.
#   P0: X[b]^T is produced on the host and DMA'd in directly.
#   P1: V = X @ Wv  ([seq, cols] layout), stored f32r with a ones column
#       appended per head (V_aug [k, 65]) so the ctx matmul also produces the
#       softmax denominator.
#   P2: per column-chunk c (= head pair 2c, 2c+1):
#       QT/KT [cols, seq] = W^T @ XT (+bq/+bk per-partition during evac).
#       The head pair occupies partitions 0-63 / 64-127, so the two heads'
#       score matmuls (contraction d=64) run concurrently in disjoint PE row
#       groups. scores^T[k, q] blocks -> ACT exp(s/8 + mask_k) straight from
#       PSUM (mask enters as the per-partition activation bias - exact).
#       ctx^T[d, q] (+denominator row) accumulates over the 16 k-blocks.
#       PE-transpose ctx^T -> [q, d], multiply by 1/denom on DVE, DMA out.
#   bv is added to the full output on the host: softmax rows sum to 1, so
#   probs @ (V0 + bv) = probs @ V0 + bv exactly (dropout prob = 0).
#
# No max-subtraction in softmax: exp(s/8 + m) at this problem's scale is far
# inside fp32 range, and large-negative masks underflow to 0 correctly.

import sys

if "/opt/trn_rl_repo" not in sys.path:
    sys.path.insert(0, "/opt/trn_rl_repo")

import numpy as np

P = 128
B, S, HID = 4, 2048, 1024
NH, HD = 16, 64
COLS = 512          # per-core slice of the hidden dim (8 heads)
HC = HID // P       # 8 hid chunks
SEQB = S // P       # 16 seq blocks (also the k blocks)
CC = COLS // P      # 4 col chunks (each = 2 heads)
QT = S // 512       # 4 q tiles of 512
KB = S // P         # 16 k blocks of 128
N_CORES = 8

_prog_cache = {}


def _build_program(repeat=1, ablate=()):
    ablate = set(ablate)
    import concourse.mybir as mybir
    from concourse import bacc
    from concourse.tile import TileContext
    from concourse.masks import make_identity

    dt = mybir.dt
    F32 = dt.float32
    F32R = dt.float32r
    BF16 = dt.bfloat16
    FP16 = dt.float16
    EXP = mybir.ActivationFunctionType.Exp
    ADD = mybir.AluOpType.add
    MULT = mybir.AluOpType.mult

    nc = bacc.Bacc(num_devices=N_CORES)

    x = nc.dram_tensor("x", [HID, S], FP16, kind="ExternalInput")  # X^T (host: fp16)
    wq = nc.dram_tensor("wq", [HID, COLS], FP16, kind="ExternalInput")
    wk = nc.dram_tensor("wk", [HID, COLS], FP16, kind="ExternalInput")
    wv = nc.dram_tensor("wv", [HID, COLS], FP16, kind="ExternalInput")
    # host pre-shapes: [128, 4] = bias[c*128 + p], [128, 16] = mask[kb*128 + p]
    bq2 = nc.dram_tensor("bq2", [P, CC], F32, kind="ExternalInput")
    bk2 = nc.dram_tensor("bk2", [P, CC], F32, kind="ExternalInput")
    mask2 = nc.dram_tensor("mask2", [P, KB], F32, kind="ExternalInput")
    out = nc.dram_tensor("out", [S, COLS], F32, kind="ExternalOutput")

    def r(ap):
        return ap.bitcast(F32R)

    def emit(tc):
        with (
            tc.tile_pool(name="persist", bufs=1) as persist,
            tc.tile_pool(name="ps_proj", bufs=2, space="PSUM") as ps_proj,
        ):
            bq_t = persist.tile([P, CC], F32, tag="bq")
            bk_t = persist.tile([P, CC], F32, tag="bk")
            mask_t = persist.tile([P, KB], F32, tag="mask")

            # XT[p, hc, s] = x[s, hc*128 + p]
            xt = persist.tile([P, HC, S], FP16, tag="xt")
            # All Wq/Wk chunks preloaded; DMA priority order: chunk-0 weights
            # and xt[0] first so the first projection matmul starts ~2us in.
            wq_t = persist.tile([P, HC, COLS], FP16, tag="wqa")
            wk_t = persist.tile([P, HC, COLS], FP16, tag="wka")
            # v_t[p, kb, h, 0:64] = V[kb*128 + p, h*64 + d]; v_t[..., 64] = 1
            v_t = persist.tile([P, KB, 8, HD + 1], FP16, tag="v")
            ones_t = persist.tile([P, 1], F32, tag="ones")
            nc.gpsimd.memset(ones_t[:], 1.0)
            nc.vector.tensor_copy(
                out=v_t[:, :, :, HD],
                in_=ones_t[:, 0, None, None].to_broadcast([P, KB, 8]),
            )

            # ---- P0: load weights + X^T (host provides x pre-transposed) ---
            # Batched DMAs (multi-dim APs): each DMA_DIRECT2D costs ~600ns of
            # SP issue time, so few big transfers beat many small ones. The
            # issue order is the critical-path order of the first Q chain:
            # wq chunk 0 (split so the first half lands sooner), x seq-block
            # 0, then wk chunk 0.
            for hh in range(2):
                hsl = slice(hh * 512, (hh + 1) * 512)
                nc.sync.dma_start(
                    wq_t[:, 4 * hh:4 * hh + 4, 0:P],
                    wq[hsl, 0:P].rearrange("(hc p) c -> p hc c", p=P),
                )
            nc.sync.dma_start(
                xt[:, :, 0:512],
                x[:, 0:512].rearrange("(hc p) s -> p hc s", p=P),
            )
            for hh in range(2):
                hsl = slice(hh * 512, (hh + 1) * 512)
                nc.sync.dma_start(
                    wk_t[:, 4 * hh:4 * hh + 4, 0:P],
                    wk[hsl, 0:P].rearrange("(hc p) c -> p hc c", p=P),
                )
            nc.sync.dma_start(bq_t[:], bq2[:])
            nc.sync.dma_start(bk_t[:], bk2[:])
            nc.sync.dma_start(mask_t[:], mask2[:])
            for s4 in range(1, QT):
                sl = slice(s4 * 512, (s4 + 1) * 512)
                nc.sync.dma_start(
                    xt[:, :, sl],
                    x[:, sl].rearrange("(hc p) s -> p hc s", p=P),
                )
            nc.sync.dma_start(
                wq_t[:, :, P:COLS],
                wq[:, P:COLS].rearrange("(hc p) c -> p hc c", p=P),
            )
            nc.sync.dma_start(
                wk_t[:, :, P:COLS],
                wk[:, P:COLS].rearrange("(hc p) c -> p hc c", p=P),
            )

            # ---- P2: per column-chunk: QK projection + attention -----------
            with (
                tc.tile_pool(name="wpool", bufs=2) as wpool,
                tc.tile_pool(name="qkpool", bufs=2) as qkpool,
                tc.tile_pool(name="exps", bufs=32) as exps_pool,
                tc.tile_pool(name="small", bufs=2) as small,
                tc.tile_pool(name="ps_sc", bufs=2, space="PSUM") as ps_sc,
                tc.tile_pool(name="ps_ctx", bufs=1, space="PSUM") as ps_ctx,
            ):
                def qk_chain(c, s4, which, qt_t, kt_t):
                    csl = slice(c * P, (c + 1) * P)
                    sl = slice(s4 * 512, (s4 + 1) * 512)
                    w_t = wq_t if which == "q" else wk_t
                    o_t = qt_t if which == "q" else kt_t
                    b_t = bq_t if which == "q" else bk_t
                    ps = ps_proj.tile([P, 512], F32, tag="proj",
                                      name=f"ps{which}_{c}_{s4}")
                    for hc in range(HC):
                        nc.tensor.matmul(
                            ps[:], w_t[:, hc, csl], xt[:, hc, sl],
                            start=(hc == 0), stop=(hc == HC - 1),
                        )
                    nc.vector.tensor_scalar(
                        o_t[:, sl], ps[:], b_t[:, c:c + 1], None, ADD
                    )

                def qk_proj(c, qt_t, kt_t):
                    for s4 in range(QT):
                        qk_chain(c, s4, "q", qt_t, kt_t)
                        qk_chain(c, s4, "k", qt_t, kt_t)

                wv_t = wpool.tile([P, HC, COLS], FP16, tag="wv", bufs=1)
                nc.sync.dma_start(
                    wv_t[:],
                    wv[:, :].rearrange("(hc p) c -> p hc c", p=P),
                )

                def v_proj_sb(sb):
                    psv = ps_proj.tile([P, COLS], F32, tag="proj",
                                       name=f"psv_{sb}")
                    for hc in range(HC):
                        nc.tensor.matmul(
                            psv[:],
                            xt[:, hc, sb * P:(sb + 1) * P],
                            wv_t[:, hc, :],
                            start=(hc == 0), stop=(hc == HC - 1),
                        )
                    nc.vector.tensor_copy(
                        out=v_t[:, sb, :, 0:HD],
                        in_=psv[:].rearrange("p (h d) -> p h d", d=HD),
                    )

                def attention(c, qt_t, kt_t, fillers=()):
                    # heads (2c, 2c+1); hsub 0 -> partitions 0:64, hsub 1 ->
                    # 64:128 (concurrent PE row groups). ctx matmuls are
                    # software-pipelined one k-block behind the score matmuls
                    # so PE always has ready work while ACT runs exp.
                    #
                    # fillers (c==0 only): remaining projection / V chains,
                    # emitted one per k-block of the first q-tile so the PE
                    # stream alternates score-pair -> chain and ACT is fed
                    # throughout the projection phase; that q-tile's ctx
                    # matmuls are deferred until V is complete (et ring of
                    # 32 keeps every exp tile alive).
                    fillers = list(fillers)
                    for q4 in range(QT):
                        qsl = slice(q4 * 512, (q4 + 1) * 512)
                        psc = [
                            ps_ctx.tile([HD + 1, 512], F32, tag=f"ctx{h}",
                                        name=f"psc_{c}_{q4}_{h}")
                            for h in range(2)
                        ]
                        exp_tiles = []

                        def ctx_mm(j):
                            if "ctx" in ablate:
                                return
                            for hsub in range(2):
                                nc.tensor.matmul(
                                    psc[hsub][:],
                                    v_t[:, j, 2 * c + hsub, :],
                                    exp_tiles[j][:, hsub, :],
                                    start=(j == 0), stop=(j == KB - 1),
                                )

                        for kb in range(KB):
                            ksl = slice(kb * P, (kb + 1) * P)
                            pss = ps_sc.tile([P, 2, 512], F32, tag="sc",
                                             name=f"pss_{c}_{q4}_{kb}")
                            if "scores" not in ablate:
                                for hsub in range(2):
                                    hp = slice(hsub * HD, hsub * HD + HD)
                                    nc.tensor.matmul(
                                        pss[:, hsub, :],
                                        kt_t[hp, ksl],
                                        qt_t[hp, qsl],
                                        start=True, stop=True,
                                    )
                            et = exps_pool.tile([P, 2, 512], FP16, tag="e",
                                                name=f"et_{c}_{q4}_{kb}")
                            if "exp" not in ablate:
                                # exp(s/8 + mask_k); mask = per-partition bias
                                nc.scalar.activation(
                                    et[:], pss[:], EXP,
                                    bias=mask_t[:, kb:kb + 1], scale=0.125,
                                )
                            exp_tiles.append(et)
                            if q4 == 0 and fillers:
                                if kb < len(fillers):
                                    fillers[kb]()
                            elif kb > 0:
                                ctx_mm(kb - 1)
                        if q4 == 0 and fillers:
                            for f in fillers[KB:]:
                                f()
                            for j in range(KB):
                                ctx_mm(j)
                        else:
                            ctx_mm(KB - 1)

                        if "tail" in ablate:
                            continue
                        ev_t = small.tile([P, 4, P], F32, tag="ev", bufs=3,
                                          name=f"ev_{c}_{q4}")
                        for hsub in range(2):
                            # ctx^T evac as fp16 (80 partitions: 65 used,
                            # padded to the 16-row xbar tile); transpose on
                            # the DMA xbar engine instead of PE.
                            ctxt = small.tile([80, 512], FP16,
                                              tag=f"ct{hsub}",
                                              name=f"ctxt_{c}_{q4}_{hsub}")
                            nc.gpsimd.memset(ctxt[HD:80, :], 0.0)
                            nc.vector.tensor_copy(
                                out=ctxt[0:HD + 1, :], in_=psc[hsub][:])
                            last = (c == CC - 1 and q4 == QT - 1)
                            for qb in range(4):
                                pstr = small.tile([P, 80], FP16, tag="ptr",
                                                  bufs=8,
                                                  name=f"pstr_{c}_{q4}_{hsub}_{qb}")
                                # each transpose trigger costs ~1.2us on its
                                # queue engine; the final block's 8 triggers
                                # have nothing to hide behind, so alternate
                                # them between the two HWDGE queues (SP and
                                # the by-then-idle ACT queue).
                                eng = nc.scalar if (last and qb % 2) else nc.sync
                                eng.dma_start_transpose(
                                    pstr[:],
                                    ctxt[:, qb * P:(qb + 1) * P],
                                )
                                rec = small.tile([P, 1], F32, tag="rec",
                                                 bufs=4,
                                                 name=f"rec_{c}_{q4}_{hsub}_{qb}")
                                nc.vector.reciprocal(rec[:], pstr[:, HD:HD + 1])
                                nc.vector.tensor_scalar(
                                    ev_t[:, qb, hsub * HD:(hsub + 1) * HD],
                                    pstr[:, 0:HD], rec[:], None, MULT,
                                )
                        nc.sync.dma_start(
                            out[q4 * 512:(q4 + 1) * 512, c * P:(c + 1) * P]
                            .rearrange("(qb p) c -> p qb c", p=P),
                            ev_t[:],
                        )

                qk_tiles = {}
                for c in range(CC):
                    qk_tiles[c] = (
                        qkpool.tile([P, S], FP16, tag="qt", name=f"qt_t_{c}"),
                        qkpool.tile([P, S], FP16, tag="kt", name=f"kt_t_{c}"),
                    )
                    qt_t, kt_t = qk_tiles[c]
                    if c == 0:
                        # Emit only the s4=0 chains up front (first score
                        # pair needs them); the rest ride as fillers. K
                        # chains lead: scores for k-block 4*s4 need kt[s4].
                        qk_chain(0, 0, "q", qt_t, kt_t)
                        qk_chain(0, 0, "k", qt_t, kt_t)
                        fillers = []
                        for s4 in range(1, QT):
                            fillers.append(
                                lambda s4=s4: qk_chain(0, s4, "k", qt_t, kt_t))
                            fillers.append(
                                lambda s4=s4: qk_chain(0, s4, "q", qt_t, kt_t))
                        for sb in range(KB):
                            fillers.append(lambda sb=sb: v_proj_sb(sb))
                        attention(c, qt_t, kt_t, fillers=fillers)
                    else:
                        qk_proj(c, qt_t, kt_t)
                        attention(c, qt_t, kt_t)

    with TileContext(nc) as tc:
        if repeat > 1:
            hints = (
                mybir.EngineType.PE, mybir.EngineType.Activation,
                mybir.EngineType.DVE, mybir.EngineType.SP,
                mybir.EngineType.Pool,
            )
            with tc.For_i(0, repeat, 1, hint_engines=hints):
                emit(tc)
        else:
            emit(tc)
    nc.compile()
    return nc


def _get_program():
    if "nc" not in _prog_cache:
        _prog_cache["nc"] = _build_program()
    return _prog_cache["nc"]


def make_in_maps(hidden_states, attention_mask, Wq, bq, Wk, bk, Wv):
    in_maps = []
    for core in range(N_CORES):
        b, half = core // 2, core % 2
        csl = slice(half * COLS, (half + 1) * COLS)
        in_maps.append({
            "x": np.ascontiguousarray(hidden_states[b].T.astype(np.float16)),
            "wq": np.ascontiguousarray(Wq[:, csl].astype(np.float16)),
            "wk": np.ascontiguousarray(Wk[:, csl].astype(np.float16)),
            "wv": np.ascontiguousarray(Wv[:, csl].astype(np.float16)),
            "bq2": np.ascontiguousarray(bq[csl].reshape(CC, P).T),
            "bk2": np.ascontiguousarray(bk[csl].reshape(CC, P).T),
            "mask2": np.ascontiguousarray(
                attention_mask[b, 0, 0, :].reshape(KB, P).T
            ),
        })
    return in_maps


def assemble_output(core_outs, bv):
    full = np.empty((B, S, HID), dtype=np.float32)
    for core in range(N_CORES):
        b, half = core // 2, core % 2
        full[b, :, half * COLS:(half + 1) * COLS] = core_outs[core]
    # exact bv handling: probs rows sum to 1 -> probs @ (V + bv) = ctx + bv
    full += np.asarray(bv, dtype=np.float32).reshape(1, 1, HID)
    return full


def kernel(hidden_states, attention_mask, Wq, bq, Wk, bk, Wv, bv):
    from concourse.bass_utils import run_bass_kernel_spmd

    hidden_states = np.asarray(hidden_states, dtype=np.float32)
    attention_mask = np.asarray(attention_mask, dtype=np.float32)
    Wq = np.asarray(Wq, dtype=np.float32)
    Wk = np.asarray(Wk, dtype=np.float32)
    Wv = np.asarray(Wv, dtype=np.float32)
    bq = np.asarray(bq, dtype=np.float32)
    bk = np.asarray(bk, dtype=np.float32)
    bv = np.asarray(bv, dtype=np.float32)

    nc = _get_program()
    in_maps = make_in_maps(hidden_states, attention_mask, Wq, bq, Wk, bk, Wv)
    res = run_bass_kernel_spmd(nc, in_maps, list(range(N_CORES)))
    return assemble_output([res.results[i]["out"] for i in range(N_CORES)], bv)

